# revision 22
# baseline (speedup 1.0000x reference)
"""Longformer (chunked sliding-window) self-attention on 8 TRN2 NeuronCores.

Sharding: sequence-parallel. B=2, L=4096 -> 8 blocks of 1024 query tokens
(4 per batch element), one block per core, each with a 512-token K/V halo
(the previous block). No cross-core communication.

v3 design (vs the v2 bf16-score baseline):
  - QKV projections run as fp8e4 DoubleRow matmuls with hi/lo error
    compensation: x and W are host-split into hi + lo (both e4m3) and the
    product takes 3 DR matmuls (hi*hi, hi*lo, lo*hi), each covering two
    128-row contraction tiles -> 0.75x the bf16 PE cost.
  - Scores are fp8e4 DoubleRow at HALF the bf16 column rate: the DR pair
    dim carries (q_hi, q_lo) on the moving side against a duplicated k_hi
    on the stationary side, so one 256-cycle DR matmul computes
    (q_hi + q_lo) . k_hi = q . k_hi exactly in q. Only the k-side e4m3
    rounding remains (~1.2e-2 end-to-end, inside the 2e-2 gate). q/k are
    stored fp8 at 64x scale straight from the projection PSUM; the 1/4096
    undo folds into the exp scale (2^-15). The k bias is dropped
    entirely: it adds a per-query constant to scores, which softmax
    cancels exactly.
  - The out-projection is fp8 DR hi/lo as well: ctx^T (bf16, from the
    DMA-xbar transpose) is split on the DVE into e4m3 hi+lo at 256x
    scale, Wo is host-split at 64x, and each [128,512] out tile takes
    12 DR matmuls (0.75x bf16). The 1/16384 undo folds into the output
    finalize. The AV ones-column is 1/256 so the reciprocal directly
    yields 256/den for the hi/lo ctx write.
  - AV stays bf16: probs come from the ACT exp per-element, so they
    cannot be hi/lo split cheaply, and plain-fp8 probs/v cost ~2.6e-2
    error. AV is reoriented: out[128 q, 65] = pT_tile.T @ [v | 1/256],
    column 64 accumulating the softmax denominator.
  - Halo keys are masked via the exp bias port: chunk-0 k-tiles 0..3 use
    a per-core [128,1] bias column (-1e9 on block-0 cores).
  - Attention PE work per exp tile is now ~1032 of 2491 cycles, so
    projection/output matmuls interleave as filler at k-tile granularity
    (deficit drainer), and each pair's AV passes are software-pipelined
    into the next pair's score/exp window.
  - Chunk 1 is processed FIRST (it reads no halo data), so the DMA
    stream delivers own-token x/weights first; all loads ride one queue
    in priority order.
  - The last chunk's output projection is two-stage: its slabs' first
    four d-tiles transpose early, the ko0..3 half accumulates mid-stream
    as filler, and the bf16 partial (+bias) is stashed in each slab's
    own retired lo half, leaving only ko4..7 plus the n1 full tiles in
    the post-attention tail.
"""

import numpy as np

B, L, D = 2, 4096, 1024
H, DH, W = 16, 64, 512
NCORES = 8
BLK = L // 4          # 1024 query tokens per core
NKV = BLK + W         # 1536 kv tokens (halo + own)
CHUNKS = BLK // W     # 2 chunks per core
KT = (2 * W) // 128   # 8 k-token tiles of 128 per chunk window

_CACHE = {}


def _build():
    import concourse.bacc as bacc
    import concourse.mybir as mybir
    import concourse.tile as tile

    f32 = mybir.dt.float32
    bf16 = mybir.dt.bfloat16
    fp8 = mybir.dt.float8e4
    AF = mybir.ActivationFunctionType
    DR = mybir.MatmulPerfMode.DoubleRow
    ALU = mybir.AluOpType

    nc = bacc.Bacc("TRN2", target_bir_lowering=False, debug=False,
                   num_devices=NCORES)

    xhi = nc.dram_tensor("xhi", [D, NKV], fp8, kind="ExternalInput").ap()
    xlo = nc.dram_tensor("xlo", [D, NKV], fp8, kind="ExternalInput").ap()
    wqh = nc.dram_tensor("wqh", [D, D], fp8, kind="ExternalInput").ap()
    wql = nc.dram_tensor("wql", [D, D], fp8, kind="ExternalInput").ap()
    wkh = nc.dram_tensor("wkh", [D, D], fp8, kind="ExternalInput").ap()
    wkl = nc.dram_tensor("wkl", [D, D], fp8, kind="ExternalInput").ap()
    wvh = nc.dram_tensor("wvh", [D, D], fp8, kind="ExternalInput").ap()
    wvl = nc.dram_tensor("wvl", [D, D], fp8, kind="ExternalInput").ap()
    woh = nc.dram_tensor("woh", [D, D], fp8, kind="ExternalInput").ap()
    wol = nc.dram_tensor("wol", [D, D], fp8, kind="ExternalInput").ap()
    bqkv = nc.dram_tensor("bqkv", [128, 17], f32, kind="ExternalInput").ap()
    borep = nc.dram_tensor("borep", [128, D], bf16,
                           kind="ExternalInput").ap()
    out = nc.dram_tensor("out", [BLK, D], f32, kind="ExternalOutput").ap()

    xhi_r = xhi.rearrange("(ko p) t -> p ko t", p=128)   # [128, 8, 1536]
    xlo_r = xlo.rearrange("(ko p) t -> p ko t", p=128)
    w_r = {n: t.rearrange("(ko p) d -> p ko d", p=128)
           for n, t in (("wqh", wqh), ("wql", wql), ("wkh", wkh),
                        ("wkl", wkl), ("wvh", wvh), ("wvl", wvl),
                        ("woh", woh), ("wol", wol))}
    out_r = out.rearrange("(to p) d -> p to d", p=128)   # [128, 8, 1024]

    # per-exp PE budget: exp [128,1024] = 1038 ns = 2491 PE cycles;
    # scores per kt (2 DR matmuls) = 512 cy, one pipelined AV half = 520.
    AV_HALF_CY = 520
    WSCALE_INV = 1.0 / 64.0
    # float8e4 here is IEEE e4m3 (max normal 240): q/k live at 32x scale
    # (max |32 q/k| ~ 140), ctx at 64x (max |64 ctx| ~ 9).
    EXP_SCALE = 0.125 / 1024.0        # 1/sqrt(dh) / (32*32 q/k scale)
    OUT_SCALE = 1.0 / 4096.0          # 1/(64 ctx scale * 64 wo scale)

    with tile.TileContext(nc) as tc:
        with (
            tc.tile_pool(name="const", bufs=1) as constp,
            tc.tile_pool(name="xw", bufs=1) as xwp,
            tc.tile_pool(name="w8", bufs=1) as w8p,
            tc.tile_pool(name="wo", bufs=1) as wop,
            tc.tile_pool(name="acts", bufs=1) as actp,
            tc.tile_pool(name="ptiles", bufs=12) as pp,
            tc.tile_pool(name="ctxn", bufs=4) as ctxp,
            tc.tile_pool(name="ctxT", bufs=2) as ctxtp,
            tc.tile_pool(name="recs", bufs=6) as recp,
            tc.tile_pool(name="outs", bufs=4) as op,
            tc.tile_pool(name="psS", bufs=2, space="PSUM") as psS,
            tc.tile_pool(name="psV", bufs=3, space="PSUM") as psV,
            tc.tile_pool(name="psA", bufs=1, space="PSUM") as psA,
        ):
            # ---- input DMA: own-token data first (chunk 1 runs first
            # and touches no halo); few, large transfers ----
            bqkv_sb = constp.tile([128, 17], f32)
            bq_sb = bqkv_sb[:, 0:8]          # 64x-scaled q bias per m-tile
            vb_sb = bqkv_sb[:, 16:17]        # exp mask bias column
            bo_sb = constp.tile([128, D], bf16)

            xh_sb = xwp.tile([128, 8, NKV], fp8, tag="xh")
            xl_sb = xwp.tile([128, 8, NKV], fp8, tag="xl")
            w_sb = {n: w8p.tile([128, 8, D], fp8, tag=n, name=n)
                    for n in ("wqh", "wql", "wkh", "wkl", "wvh", "wvl")}
            def wload(n, c0, c1):
                nc.sync.dma_start(w_sb[n][:, :, c0:c1], w_r[n][:, :, c0:c1])

            wload("wqh", 0, 128)
            nc.sync.dma_start(xh_sb[:, :, 1024:1536], xhi_r[:, :, 1024:1536])
            wload("wkh", 0, 128)
            wload("wkl", 0, 128)
            nc.sync.dma_start(xh_sb[:, :, 512:1024], xhi_r[:, :, 512:1024])
            wload("wql", 0, 128)
            nc.sync.dma_start(xl_sb[:, :, 1024:1536], xlo_r[:, :, 1024:1536])
            nc.sync.dma_start(xl_sb[:, :, 512:1024], xlo_r[:, :, 512:1024])
            # one full-width transfer per weight: a 128:512 column slice has
            # 384B contiguous runs and pays the <512B 2x DMA latency, while
            # 128:1024 runs at 896B/descriptor.
            nc.sync.dma_start(bqkv_sb[:], bqkv[:])
            wload("wqh", 128, 1024)
            wload("wql", 128, 1024)
            wload("wkh", 128, 1024)
            wload("wkl", 128, 1024)
            wload("wvh", 0, 1024)
            wload("wvl", 0, 1024)
            nc.sync.dma_start(bo_sb[:], borep[:])
            nc.sync.dma_start(xh_sb[:, :, 0:512], xhi_r[:, :, 0:512])
            nc.sync.dma_start(xl_sb[:, :, 0:512], xlo_r[:, :, 0:512])

            # ---- persistent activations ----
            # q/k fp8 at 32x scale; q dim2 = DR pair slot (hi, lo); k is
            # stored once and stride-0 broadcast across the DR pair.
            q_sb = actp.tile([128, 8, 2, BLK], fp8, tag="q")
            k_sb = actp.tile([128, 8, NKV], fp8, tag="k")
            # v natural [tok, h, dh+1]; col 64 per head = 1/64 column so
            # AV's denominator lands pre-scaled for the 64x ctx split.
            v_sb = actp.tile([128, 12, H * (DH + 1)], bf16, tag="v")
            v_v = v_sb[:].rearrange("p t (h e) -> p t h e", e=DH + 1)
            nc.vector.memset(v_v[:, :, :, DH], 1.0 / 64.0)
            # ctx^T fp8 hi/lo (256x scale) for the DR out-projection; the
            # bf16 transpose destination is a small rotating piece pool.
            ct8h_sb = actp.tile([128, 8, BLK], fp8, tag="ct8h")
            ct8l_sb = actp.tile([128, 8, BLK], fp8, tag="ct8l")
            wo8h_sb = wop.tile([128, 8, D], fp8, tag="wo8h", name="wo8h")
            wo8l_sb = wop.tile([128, 8, D], fp8, tag="wo8l", name="wo8l")

            # ---- projections (fp8 DoubleRow hi/lo) ----
            ps_rot = [psA, psV]
            rot_i = [0]

            def dr_group(ps, lhs_hi, lhs_lo, rhs_hi, rhs_lo, lsl, rsl):
                """12 DR matmuls: hi*hi (4 ko-pair steps) + hi*lo + lo*hi."""
                for term, (lh, rh) in enumerate(
                        ((lhs_hi, rhs_hi), (lhs_hi, rhs_lo), (lhs_lo, rhs_hi))):
                    for s in range(4):
                        ksl = slice(2 * s, 2 * s + 2)
                        nc.tensor.matmul(
                            ps[:], lh[:, ksl, lsl], rh[:, ksl, rsl],
                            start=(term == 0 and s == 0),
                            stop=(term == 2 and s == 3), perf_mode=DR)

            def proj_ps(borrow=True):
                pool = ps_rot[rot_i[0] % 2]
                rot_i[0] += 1
                tag = "ps" if pool is psA else "av"
                return pool.tile([128, 512], f32, name="bps", tag=tag)

            def qk_psum(wh, wl, xn, m):
                ps = proj_ps(False)
                dr_group(ps, w_sb[wh], w_sb[wl], xh_sb[:], xl_sb[:],
                         slice(m * 128, (m + 1) * 128),
                         slice(xn * 512, (xn + 1) * 512))
                return ps

            def q_fin(ps, m, dn):
                dsl = slice(dn * 512, dn * 512 + 512)
                nc.vector.tensor_scalar_add(
                    q_sb[:, m, 0, dsl], ps[:], bq_sb[:, m:m + 1])
                nc.vector.scalar_tensor_tensor(
                    q_sb[:, m, 1, dsl], ps[:], bq_sb[:, m:m + 1],
                    q_sb[:, m, 0, dsl], ALU.add, ALU.subtract)

            def k_fin(ps, m, dn):
                dsl = slice(dn * 512, dn * 512 + 512)
                nc.vector.tensor_copy(k_sb[:, m, dsl], ps[:])

            def proj_q_m(xn, dn, m):
                q_fin(qk_psum("wqh", "wql", xn, m), m, dn)

            def proj_k_m(xn, dn, m):
                k_fin(qk_psum("wkh", "wkl", xn, m), m, dn)

            def proj_v_t(t, n, borrow=False):
                """One [128 tok x 512 feature] tile of the v projection."""
                ps = proj_ps(borrow)
                dr_group(ps, xh_sb[:], xl_sb[:], w_sb["wvh"], w_sb["wvl"],
                         slice(t * 128, (t + 1) * 128),
                         slice(n * 512, (n + 1) * 512))
                nc.vector.tensor_scalar_mul(
                    v_v[:, t, n * 8:(n + 1) * 8, :DH],
                    ps[:].rearrange("p (h e) -> p h e", e=DH), WSCALE_INV)

            # ---- filler: PE work interleaved into the attention stream ----
            filler = []
            state = {"deficit": 0, "idx": 0}

            def add_filler(cycles, fn):
                filler.append((cycles, fn))

            def drain(cycles):
                state["deficit"] += cycles
                while (state["idx"] < len(filler)
                       and state["deficit"] >= filler[state["idx"]][0]):
                    cyc, fn = filler[state["idx"]]
                    state["idx"] += 1
                    state["deficit"] -= cyc
                    fn()

            def drain_to(idx):
                while state["idx"] < min(idx, len(filler)):
                    state["deficit"] = 0
                    cyc, fn = filler[state["idx"]]
                    state["idx"] += 1
                    fn()

            # ---- attention ----
            ctx_slabs = {}

            def ct8_split(src, kolo, kohi, csl):
                """fp8 hi/lo split of a transposed ctxT piece (the slab is
                already at 64x scale via the 1/64 AV ones-column)."""
                nc.vector.tensor_copy(ct8h_sb[:, kolo:kohi, csl], src)
                nc.vector.scalar_tensor_tensor(
                    ct8l_sb[:, kolo:kohi, csl], src, 1.0,
                    ct8h_sb[:, kolo:kohi, csl], ALU.mult, ALU.subtract)

            def transpose(c, qt):
                csl = slice((c * 4 + qt) * 128, (c * 4 + qt + 1) * 128)
                t = ctxtp.tile([128, 8, 128], bf16, tag="ctp", name="ctp")
                nc.sync.dma_start_transpose(t[:], ctx_slabs[(c, qt)][:])
                ct8_split(t[:], 0, 8, csl)

            KSPLIT = 4   # ko 0..KSPLIT-1 transposed early, rest in tail

            def transpose_half(c, qt, half):
                lo, hi = (0, KSPLIT) if half == 0 else (KSPLIT, 8)
                csl = slice((c * 4 + qt) * 128, (c * 4 + qt + 1) * 128)
                t = ctxtp.tile([128, 8, 128], bf16, tag="ctp", name="ctp")
                nc.sync.dma_start_transpose(
                    t[:, lo:hi, :],
                    ctx_slabs[(c, qt)][:, lo * 128:hi * 128])
                ct8_split(t[:, lo:hi, :], lo, hi, csl)

            def out_part(to, n):
                # dead bf16 storage for chunk-0 partials: n0 in the slab's
                # own lo half (read-complete after its early transpose), n1
                # in retired q8 m0/m1 slabs viewed as bf16.
                if n == 0:
                    return ctx_slabs[(0, to)][:, 0:512]
                m, half = divmod(to, 2)
                qv = q_sb[:, m, :, :].rearrange(
                    "p a b -> p (a b)").bitcast(bf16)
                return qv[:, half * 512:(half + 1) * 512]

            def out_mm(ps, to, n, ko0, ko1):
                """DR out-proj matmuls over ko tiles [ko0, ko1)."""
                tsl = slice(to * 128, (to + 1) * 128)
                nsl = slice(n * 512, (n + 1) * 512)
                steps = [(t, s) for t in range(3)
                         for s in range(ko0 // 2, ko1 // 2)]
                mats = ((ct8h_sb, wo8h_sb), (ct8h_sb, wo8l_sb),
                        (ct8l_sb, wo8h_sb))
                for i, (t, s) in enumerate(steps):
                    lh, rh = mats[t]
                    ksl = slice(2 * s, 2 * s + 2)
                    nc.tensor.matmul(
                        ps[:], lh[:, ksl, tsl], rh[:, ksl, nsl],
                        start=(i == 0), stop=(i == len(steps) - 1),
                        perf_mode=DR)

            def out_stage1(to, n):
                # chunk-0: accumulate ko 0..KSPLIT-1 mid-stream, stash the
                # bf16 partial (+bias) in dead storage
                ps = proj_ps()
                out_mm(ps, to, n, 0, KSPLIT)
                nc.vector.scalar_tensor_tensor(
                    out_part(to, n), ps[:], OUT_SCALE,
                    bo_sb[:, n * 512:(n + 1) * 512], ALU.mult, ALU.add)

            def out_stage2(to, n):
                ps = proj_ps()
                out_mm(ps, to, n, KSPLIT, 8)
                o_t = op.tile([128, 512], f32, tag="o", name="o")
                nc.vector.scalar_tensor_tensor(
                    o_t[:], ps[:], OUT_SCALE, out_part(to, n),
                    ALU.mult, ALU.add)
                nc.sync.dma_start(out_r[:, to, n * 512:(n + 1) * 512], o_t[:])

            PASSES = ((0, 0, 1), (1, 0, 1), (0, 2, 3), (1, 2, 3))
            pending = [None]      # (c, u, plist, slot, tiles) awaiting AV
            pair_no = [0]

            def av_half(c, u, plist, pass_i, half, tiles):
                """Half an AV pass (kt 4*half..4*half+3) of heads 2u+g."""
                g, qta, qtb = PASSES[pass_i]
                h = 2 * u + g
                for kt in range(4 * half, 4 * half + 4):
                    vsl = v_v[:, c * 4 + kt, h, :]
                    for qt, av in zip((qta, qtb), tiles):
                        nc.tensor.matmul(
                            av[:, 0:65],
                            plist[kt][:, g * 512 + qt * 128:
                                      g * 512 + (qt + 1) * 128],
                            vsl, start=(kt == 0), stop=(kt == KT - 1))
                if half == 1:
                    for qt, av in zip((qta, qtb), tiles):
                        rec = recp.tile([128, 1], f32, tag="rec", name="rec")
                        nc.vector.reciprocal(rec[:], av[:, 64:65])
                        nc.vector.tensor_scalar_mul(
                            ctx_slabs[(c, qt)][:, h * 64:(h + 1) * 64],
                            av[:, 0:64], rec[:, 0:1])

            def flush_slot(s=None):
                """Emit AV half-slot s (or all remaining) of the pending
                pair; after the chunk's last pair, transposes chase."""
                if pending[0] is None:
                    return
                c, u, plist, done, tiles = pending[0]
                rng = range(8) if s is None else [s]
                for s_ in rng:
                    if s_ < done:
                        continue
                    pass_i, half = divmod(s_, 2)
                    if half == 0:
                        tiles = [psV.tile([128, 512], f32, name="av",
                                          tag="av") for _ in range(2)]
                    av_half(c, u, plist, pass_i, half, tiles)
                    pending[0] = (c, u, plist, s_ + 1, tiles)
                    if half == 0:
                        continue
                    if c == 0 and u == KSPLIT - 1 and pass_i == 3:
                        for qt in range(4):
                            transpose_half(0, qt, 0)
                        for to in range(4):
                            add_filler(1536,
                                       lambda to=to: out_stage1(to, 0))
                        for to in range(4):
                            add_filler(1536,
                                       lambda to=to: out_stage1(to, 1))
                    if u == 7:
                        if pass_i == 1:
                            if c == 0:
                                transpose_half(c, 0, 1)
                                transpose_half(c, 1, 1)
                            else:
                                transpose(c, 0)
                                transpose(c, 1)
                        if pass_i == 3:
                            if c == 0:
                                transpose_half(c, 2, 1)
                                transpose_half(c, 3, 1)
                            else:
                                transpose(c, 2)
                                transpose(c, 3)
                if pending[0][3] >= 8:
                    pending[0] = None

            def emit_pair(c, u, force_idx=None):
                if force_idx is not None:
                    drain_to(force_idx)
                plist = []
                for kt in range(KT):
                    ksl = slice(c * 512 + kt * 128, c * 512 + (kt + 1) * 128)
                    qsl = slice(c * 512, (c + 1) * 512)
                    sps = psS.tile([128, 1024], f32, name="sps")
                    kA = k_sb[0:64, u, ksl].unsqueeze(1).broadcast_to(
                        [64, 2, 128])
                    kB = k_sb[64:128, u, ksl].unsqueeze(1).broadcast_to(
                        [64, 2, 128])
                    nc.tensor.matmul(sps[:, 0:512], kA,
                                     q_sb[0:64, u, :, qsl], start=True,
                                     stop=True, perf_mode=DR)
                    nc.tensor.matmul(sps[:, 512:1024], kB,
                                     q_sb[64:128, u, :, qsl], start=True,
                                     stop=True, perf_mode=DR)
                    p_t = pp.tile([128, 1024], bf16, tag="p", name="p")
                    if c == 0 and kt < 4:
                        nc.scalar.activation(p_t[:], sps[:], AF.Exp,
                                             bias=vb_sb[:, 0:1],
                                             scale=EXP_SCALE)
                    else:
                        nc.scalar.activation(p_t[:], sps[:], AF.Exp,
                                             scale=EXP_SCALE)
                    plist.append(p_t)
                    # drain below the exp-paced slack (2491-512-520 cy/kt)
                    # so the PE can repay startup DMA debt; the excess PE
                    # work was front-loaded pre-stream.
                    budget = 1600
                    if pending[0] is not None:
                        pc, pu = pending[0][0], pending[0][1]
                        if kt < 2 and (pc, pu, kt) in av_force:
                            drain_to(av_force[(pc, pu, kt)])
                        flush_slot(kt)
                        budget -= AV_HALF_CY
                    drain(max(budget, 0))
                flush_slot()   # no-op unless fewer than 8 slots were free
                pending[0] = (c, u, plist, 0, None)
                pair_no[0] += 1

            def out_proj(to, n, borrow=False):
                ps = proj_ps(borrow)
                out_mm(ps, to, n, 0, 8)
                o_t = op.tile([128, 512], f32, tag="o", name="o")
                nc.vector.scalar_tensor_tensor(
                    o_t[:], ps[:], OUT_SCALE,
                    bo_sb[:, n * 512:(n + 1) * 512], ALU.mult, ALU.add)
                nc.sync.dma_start(out_r[:, to, n * 512:(n + 1) * 512], o_t[:])

            # ---- phase schedule ----
            # Chunk 1 first: it touches no halo data, so the DMA stream
            # delivers own-token x/w first and the halo trails.
            nc.sync.dma_start(wo8h_sb[:], w_r["woh"][:])
            nc.sync.dma_start(wo8l_sb[:], w_r["wol"][:])

            # pre-attention: just what (1,0)'s first score tiles need,
            # as two interleaved m=0 groups ordered so the xl-dependent
            # cross terms come last (xl is the longest DMA pole).
            psQ = psA.tile([128, 512], f32, name="bps", tag="ps")
            psK = psV.tile([128, 512], f32, name="bps", tag="av")
            m0 = slice(0, 128)
            for ps, wh, xn in ((psQ, "wqh", 2), (psK, "wkh", 1)):
                xs = slice(xn * 512, (xn + 1) * 512)
                for s4 in range(4):
                    ks = slice(2 * s4, 2 * s4 + 2)
                    nc.tensor.matmul(ps[:], w_sb[wh][:, ks, m0],
                                     xh_sb[:, ks, xs],
                                     start=(s4 == 0), stop=False, perf_mode=DR)
            for ps, wl, xn in ((psQ, "wql", 2), (psK, "wkl", 1)):
                xs = slice(xn * 512, (xn + 1) * 512)
                for s4 in range(4):
                    ks = slice(2 * s4, 2 * s4 + 2)
                    nc.tensor.matmul(ps[:], w_sb[wl][:, ks, m0],
                                     xh_sb[:, ks, xs],
                                     start=False, stop=False, perf_mode=DR)
            for ps, wh, xn in ((psQ, "wqh", 2), (psK, "wkh", 1)):
                xs = slice(xn * 512, (xn + 1) * 512)
                for s4 in range(4):
                    ks = slice(2 * s4, 2 * s4 + 2)
                    nc.tensor.matmul(ps[:], w_sb[wh][:, ks, m0],
                                     xl_sb[:, ks, xs],
                                     start=False, stop=(s4 == 3), perf_mode=DR)
            q_fin(psQ, 0, 1)
            k_fin(psK, 0, 1)

            score_force = {}
            av_force = {}
            add_filler(3072, lambda: proj_k_m(2, 2, 0))

            def add_qk1(u):
                add_filler(3072, lambda m=u: proj_q_m(2, 1, m))
                add_filler(3072, lambda m=u: proj_k_m(1, 1, m))
                add_filler(3072, lambda m=u: proj_k_m(2, 2, m))
                score_force[(1, u)] = len(filler)

            def add_qk0(u):
                add_filler(3072, lambda m=u: proj_q_m(1, 0, m))
                add_filler(3072, lambda m=u: proj_k_m(0, 0, m))
                score_force[(0, u)] = len(filler)

            add_qk1(1)
            for t in range(4, 8):
                add_filler(3072, lambda t=t: proj_v_t(t, 0))
            av_force[(1, 0, 0)] = len(filler)
            for t in range(8, 12):
                add_filler(3072, lambda t=t: proj_v_t(t, 0))
            av_force[(1, 0, 1)] = len(filler)
            add_qk1(2)
            add_qk1(3)
            add_qk1(4)
            for t in range(4, 8):
                add_filler(3072, lambda t=t: proj_v_t(t, 1))
            av_force[(1, 4, 0)] = len(filler)
            for t in range(8, 12):
                add_filler(3072, lambda t=t: proj_v_t(t, 1))
            av_force[(1, 4, 1)] = len(filler)
            for u in (5, 6, 7):
                add_qk1(u)
            for u in range(8):
                add_qk0(u)
            for t in range(4):
                add_filler(3072, lambda t=t: proj_v_t(t, 0))
            av_force[(0, 0, 0)] = len(filler)
            for t in range(4):
                add_filler(3072, lambda t=t: proj_v_t(t, 1))
            av_force[(0, 4, 0)] = len(filler)
            for to in range(4, 8):
                for n in range(2):
                    add_filler(3072, lambda to=to, n=n: out_proj(to, n))

            # front-load the PE excess (the stream can only absorb filler
            # at ~1100 cy/kt with catch-up slack): run k2-m0 + qk1(1..4) +
            # the v-n0 batch inline pre-stream, paced by the DMA arrivals.
            drain_to(21)

            for qt in range(4):
                ctx_slabs[(1, qt)] = ctxp.tile([128, BLK], bf16, tag="slab",
                                               name="slab")
            for u in range(8):
                emit_pair(1, u, force_idx=score_force.get((1, u)))
            for qt in range(4):
                ctx_slabs[(0, qt)] = ctxp.tile([128, BLK], bf16, tag="slab",
                                               name="slab")
            for u in range(8):
                emit_pair(0, u, force_idx=score_force.get((0, u)))

            flush_slot()
            drain_to(len(filler))
            for to in range(4):
                out_stage2(to, 0)
                out_stage2(to, 1)

    nc.compile()
    return nc


def _host_prep(x, Wq, bq, Wk, bk, Wv, bv, Wo, bo):
    import ml_dtypes

    e4 = ml_dtypes.float8_e4m3
    bf = ml_dtypes.bfloat16

    def split8(a):
        a = np.ascontiguousarray(a, dtype=np.float32)
        hi = a.astype(e4)
        lo = (a - hi.astype(np.float32)).astype(e4)
        return hi, lo

    x = np.ascontiguousarray(np.asarray(x, dtype=np.float32))
    Wq = np.asarray(Wq, np.float32)
    Wk = np.asarray(Wk, np.float32)
    Wv = np.asarray(Wv, np.float32)
    Wo = np.asarray(Wo, np.float32)
    bv = np.asarray(bv, np.float32)
    bo = np.asarray(bo, np.float32)

    # q/k weights at 32x so the fp8 q/k (psum scale) stay under e4m3's
    # 240 max; v/o weights at 64x as before.
    wqh, wql = split8(Wq.T * 32.0)
    wkh, wkl = split8(Wk.T * 32.0)
    wvh, wvl = split8(Wv.T * 64.0)
    woh, wol = split8(Wo.T * 64.0)
    bo_eff = bo + Wo @ bv          # v-bias folded through the attention avg
    mats = {
        "wqh": wqh, "wql": wql, "wkh": wkh, "wkl": wkl,
        "wvh": wvh, "wvl": wvl, "woh": woh, "wol": wol,
        "borep": np.ascontiguousarray(
            np.tile(bo_eff[None, :], (128, 1)).astype(bf)),
    }
    bqkv = np.zeros((128, 17), np.float32)
    # q bias at the 32x q scale; k bias is dropped (softmax-invariant).
    bqkv[:, 0:8] = np.asarray(bq, np.float32).reshape(8, 128).T * 32.0
    mats["bqkv_base"] = bqkv

    in_maps = []
    for core in range(NCORES):
        b, j = core // 4, core % 4
        start = j * BLK
        xkv = np.zeros((NKV, D), np.float32)
        lo = start - W
        if lo < 0:
            xkv[W:] = x[b, start:start + BLK]
        else:
            xkv[:] = x[b, lo:start + BLK]
        xh, xl = split8(xkv.T)
        im = dict(mats)
        bqkv_c = mats["bqkv_base"].copy()
        if j == 0:
            bqkv_c[:, 16] = -1e9   # chunk-0 halo k-tiles masked in the exp
        del im["bqkv_base"]
        im["bqkv"] = np.ascontiguousarray(bqkv_c)
        im["xhi"] = xh
        im["xlo"] = xl
        in_maps.append(im)
    return in_maps


def kernel(x, Wq, bq, Wk, bk, Wv, bv, Wo, bo):
    from concourse.bass_utils import run_bass_kernel_spmd

    if "nc" not in _CACHE:
        _CACHE["nc"] = _build()
    nc = _CACHE["nc"]

    in_maps = _host_prep(x, Wq, bq, Wk, bk, Wv, bv, Wo, bo)
    res = run_bass_kernel_spmd(nc, in_maps, list(range(NCORES)))

    out = np.empty((B, L, D), np.float32)
    for core in range(NCORES):
        b, j = core // 4, core % 4
        out[b, j * BLK:(j + 1) * BLK] = res.results[core]["out"]
    return out


# revision 23
# speedup vs baseline: 1.0537x; 1.0537x over previous
"""Longformer (chunked sliding-window) self-attention on 8 TRN2 NeuronCores.

Sharding: sequence-parallel. B=2, L=4096 -> 8 blocks of 1024 query tokens
(4 per batch element), one block per core, each with a 512-token K/V halo
(the previous block). No cross-core communication.

v3 design (vs the v2 bf16-score baseline):
  - QKV projections run as fp8e4 DoubleRow matmuls with hi/lo error
    compensation: x and W are host-split into hi + lo (both e4m3) and the
    product takes 3 DR matmuls (hi*hi, hi*lo, lo*hi), each covering two
    128-row contraction tiles -> 0.75x the bf16 PE cost.
  - Scores are fp8e4 DoubleRow at HALF the bf16 column rate: the DR pair
    dim carries (q_hi, q_lo) on the moving side against a duplicated k_hi
    on the stationary side, so one 256-cycle DR matmul computes
    (q_hi + q_lo) . k_hi = q . k_hi exactly in q. Only the k-side e4m3
    rounding remains (~1.2e-2 end-to-end, inside the 2e-2 gate). q/k are
    stored fp8 at 64x scale straight from the projection PSUM; the 1/4096
    undo folds into the exp scale (2^-15). The k bias is dropped
    entirely: it adds a per-query constant to scores, which softmax
    cancels exactly.
  - The out-projection is fp8 DR hi/lo as well: ctx^T (bf16, from the
    DMA-xbar transpose) is split on the DVE into e4m3 hi+lo at 256x
    scale, Wo is host-split at 64x, and each [128,512] out tile takes
    12 DR matmuls (0.75x bf16). The 1/16384 undo folds into the output
    finalize. The AV ones-column is 1/256 so the reciprocal directly
    yields 256/den for the hi/lo ctx write.
  - AV stays bf16: probs come from the ACT exp per-element, so they
    cannot be hi/lo split cheaply, and plain-fp8 probs/v cost ~2.6e-2
    error. AV is reoriented: out[128 q, 65] = pT_tile.T @ [v | 1/256],
    column 64 accumulating the softmax denominator.
  - Halo keys are masked via the exp bias port: chunk-0 k-tiles 0..3 use
    a per-core [128,1] bias column (-1e9 on block-0 cores).
  - Attention PE work per exp tile is now ~1032 of 2491 cycles, so
    projection/output matmuls interleave as filler at k-tile granularity
    (deficit drainer), and each pair's AV passes are software-pipelined
    into the next pair's score/exp window.
  - Chunk 1 is processed FIRST (it reads no halo data), so the DMA
    stream delivers own-token x/weights first; all loads ride one queue
    in priority order.
  - The last chunk's output projection is two-stage: its slabs' first
    four d-tiles transpose early, the ko0..3 half accumulates mid-stream
    as filler, and the bf16 partial (+bias) is stashed in each slab's
    own retired lo half, leaving only ko4..7 plus the n1 full tiles in
    the post-attention tail.
"""

import numpy as np

B, L, D = 2, 4096, 1024
H, DH, W = 16, 64, 512
NCORES = 8
BLK = L // 4          # 1024 query tokens per core
NKV = BLK + W         # 1536 kv tokens (halo + own)
CHUNKS = BLK // W     # 2 chunks per core
KT = (2 * W) // 128   # 8 k-token tiles of 128 per chunk window

_CACHE = {}


def _build():
    import concourse.bacc as bacc
    import concourse.mybir as mybir
    import concourse.tile as tile

    f32 = mybir.dt.float32
    bf16 = mybir.dt.bfloat16
    fp8 = mybir.dt.float8e4
    AF = mybir.ActivationFunctionType
    DR = mybir.MatmulPerfMode.DoubleRow
    ALU = mybir.AluOpType

    nc = bacc.Bacc("TRN2", target_bir_lowering=False, debug=False,
                   num_devices=NCORES)

    xhi = nc.dram_tensor("xhi", [D, NKV], fp8, kind="ExternalInput").ap()
    xlo = nc.dram_tensor("xlo", [D, NKV], fp8, kind="ExternalInput").ap()
    wqh = nc.dram_tensor("wqh", [D, D], fp8, kind="ExternalInput").ap()
    wql = nc.dram_tensor("wql", [D, D], fp8, kind="ExternalInput").ap()
    wkh = nc.dram_tensor("wkh", [D, D], fp8, kind="ExternalInput").ap()
    wkl = nc.dram_tensor("wkl", [D, D], fp8, kind="ExternalInput").ap()
    wvh = nc.dram_tensor("wvh", [D, D], fp8, kind="ExternalInput").ap()
    wvl = nc.dram_tensor("wvl", [D, D], fp8, kind="ExternalInput").ap()
    woh = nc.dram_tensor("woh", [D, D], fp8, kind="ExternalInput").ap()
    wol = nc.dram_tensor("wol", [D, D], fp8, kind="ExternalInput").ap()
    bqkv = nc.dram_tensor("bqkv", [128, 17], f32, kind="ExternalInput").ap()
    borep = nc.dram_tensor("borep", [128, D], bf16,
                           kind="ExternalInput").ap()
    out = nc.dram_tensor("out", [BLK, D], f32, kind="ExternalOutput").ap()

    xhi_r = xhi.rearrange("(ko p) t -> p ko t", p=128)   # [128, 8, 1536]
    xlo_r = xlo.rearrange("(ko p) t -> p ko t", p=128)
    w_r = {n: t.rearrange("(ko p) d -> p ko d", p=128)
           for n, t in (("wqh", wqh), ("wql", wql), ("wkh", wkh),
                        ("wkl", wkl), ("wvh", wvh), ("wvl", wvl),
                        ("woh", woh), ("wol", wol))}
    out_r = out.rearrange("(to p) d -> p to d", p=128)   # [128, 8, 1024]

    # per-exp PE budget: exp [128,1024] = 1038 ns = 2491 PE cycles;
    # scores per kt (2 DR matmuls) = 512 cy, one pipelined AV half = 520.
    AV_HALF_CY = 520
    WSCALE_INV = 1.0 / 64.0
    # float8e4 here is IEEE e4m3 (max normal 240): q/k live at 32x scale
    # (max |32 q/k| ~ 140), ctx at 64x (max |64 ctx| ~ 9).
    EXP_SCALE = 0.125 / 1024.0        # 1/sqrt(dh) / (32*32 q/k scale)
    OUT_SCALE = 1.0 / 4096.0          # 1/(64 ctx scale * 64 wo scale)

    with tile.TileContext(nc) as tc:
        with (
            tc.tile_pool(name="const", bufs=1) as constp,
            tc.tile_pool(name="xw", bufs=1) as xwp,
            tc.tile_pool(name="w8", bufs=1) as w8p,
            tc.tile_pool(name="wo", bufs=1) as wop,
            tc.tile_pool(name="acts", bufs=1) as actp,
            tc.tile_pool(name="ptiles", bufs=12) as pp,
            tc.tile_pool(name="ctxn", bufs=4) as ctxp,
            tc.tile_pool(name="ctxT", bufs=2) as ctxtp,
            tc.tile_pool(name="recs", bufs=6) as recp,
            tc.tile_pool(name="outs", bufs=4) as op,
            tc.tile_pool(name="psS", bufs=2, space="PSUM") as psS,
            tc.tile_pool(name="psV", bufs=3, space="PSUM") as psV,
            tc.tile_pool(name="psA", bufs=1, space="PSUM") as psA,
        ):
            # ---- input DMA: own-token data first (chunk 1 runs first
            # and touches no halo); few, large transfers ----
            bqkv_sb = constp.tile([128, 17], f32)
            bq_sb = bqkv_sb[:, 0:8]          # 64x-scaled q bias per m-tile
            vb_sb = bqkv_sb[:, 16:17]        # exp mask bias column
            bo_sb = constp.tile([128, D], bf16)

            xh_sb = xwp.tile([128, 8, NKV], fp8, tag="xh")
            xl_sb = xwp.tile([128, 8, NKV], fp8, tag="xl")
            w_sb = {n: w8p.tile([128, 8, D], fp8, tag=n, name=n)
                    for n in ("wqh", "wql", "wkh", "wkl", "wvh", "wvl")}
            def wload(n, c0, c1):
                nc.sync.dma_start(w_sb[n][:, :, c0:c1], w_r[n][:, :, c0:c1])

            wload("wqh", 0, 128)
            nc.sync.dma_start(xh_sb[:, :, 1024:1536], xhi_r[:, :, 1024:1536])
            wload("wkh", 0, 128)
            wload("wkl", 0, 128)
            nc.sync.dma_start(xh_sb[:, :, 512:1024], xhi_r[:, :, 512:1024])
            wload("wql", 0, 128)
            nc.sync.dma_start(xl_sb[:, :, 1024:1536], xlo_r[:, :, 1024:1536])
            nc.sync.dma_start(xl_sb[:, :, 512:1024], xlo_r[:, :, 512:1024])
            # the pre-stream front-load needs wq/wk m1 + wv n0 first (small
            # slices), then the remainders ride as wide >=512B-run transfers
            # (a <512B contiguous run pays 2x DMA latency per descriptor).
            nc.sync.dma_start(bqkv_sb[:], bqkv[:])
            wload("wqh", 128, 256)
            wload("wql", 128, 256)
            wload("wkh", 128, 256)
            wload("wkl", 128, 256)
            wload("wvh", 0, 512)
            wload("wvl", 0, 512)
            wload("wqh", 256, 1024)
            wload("wql", 256, 1024)
            wload("wkh", 256, 1024)
            wload("wkl", 256, 1024)
            wload("wvh", 512, 1024)
            wload("wvl", 512, 1024)
            nc.sync.dma_start(bo_sb[:], borep[:])
            nc.sync.dma_start(xh_sb[:, :, 0:512], xhi_r[:, :, 0:512])
            nc.sync.dma_start(xl_sb[:, :, 0:512], xlo_r[:, :, 0:512])

            # ---- persistent activations ----
            # q/k fp8 at 32x scale; q dim2 = DR pair slot (hi, lo); k is
            # stored once and stride-0 broadcast across the DR pair.
            q_sb = actp.tile([128, 8, 2, BLK], fp8, tag="q")
            k_sb = actp.tile([128, 8, NKV], fp8, tag="k")
            # v natural [tok, h, dh+1]; col 64 per head = 1/64 column so
            # AV's denominator lands pre-scaled for the 64x ctx split.
            v_sb = actp.tile([128, 12, H * (DH + 1)], bf16, tag="v")
            v_v = v_sb[:].rearrange("p t (h e) -> p t h e", e=DH + 1)
            nc.vector.memset(v_v[:, :, :, DH], 1.0 / 64.0)
            # ctx^T fp8 hi/lo (256x scale) for the DR out-projection; the
            # bf16 transpose destination is a small rotating piece pool.
            ct8h_sb = actp.tile([128, 8, BLK], fp8, tag="ct8h")
            ct8l_sb = actp.tile([128, 8, BLK], fp8, tag="ct8l")
            wo8h_sb = wop.tile([128, 8, D], fp8, tag="wo8h", name="wo8h")
            wo8l_sb = wop.tile([128, 8, D], fp8, tag="wo8l", name="wo8l")

            # ---- projections (fp8 DoubleRow hi/lo) ----
            ps_rot = [psA, psV]
            rot_i = [0]

            def dr_group(ps, lhs_hi, lhs_lo, rhs_hi, rhs_lo, lsl, rsl):
                """12 DR matmuls: hi*hi (4 ko-pair steps) + hi*lo + lo*hi."""
                for term, (lh, rh) in enumerate(
                        ((lhs_hi, rhs_hi), (lhs_hi, rhs_lo), (lhs_lo, rhs_hi))):
                    for s in range(4):
                        ksl = slice(2 * s, 2 * s + 2)
                        nc.tensor.matmul(
                            ps[:], lh[:, ksl, lsl], rh[:, ksl, rsl],
                            start=(term == 0 and s == 0),
                            stop=(term == 2 and s == 3), perf_mode=DR)

            def proj_ps(borrow=True):
                pool = ps_rot[rot_i[0] % 2]
                rot_i[0] += 1
                tag = "ps" if pool is psA else "av"
                return pool.tile([128, 512], f32, name="bps", tag=tag)

            def qk_psum(wh, wl, xn, m):
                ps = proj_ps(False)
                dr_group(ps, w_sb[wh], w_sb[wl], xh_sb[:], xl_sb[:],
                         slice(m * 128, (m + 1) * 128),
                         slice(xn * 512, (xn + 1) * 512))
                return ps

            def q_fin(ps, m, dn):
                dsl = slice(dn * 512, dn * 512 + 512)
                nc.vector.tensor_scalar_add(
                    q_sb[:, m, 0, dsl], ps[:], bq_sb[:, m:m + 1])
                nc.vector.scalar_tensor_tensor(
                    q_sb[:, m, 1, dsl], ps[:], bq_sb[:, m:m + 1],
                    q_sb[:, m, 0, dsl], ALU.add, ALU.subtract)

            def k_fin(ps, m, dn):
                dsl = slice(dn * 512, dn * 512 + 512)
                nc.vector.tensor_copy(k_sb[:, m, dsl], ps[:])

            def proj_q_m(xn, dn, m):
                q_fin(qk_psum("wqh", "wql", xn, m), m, dn)

            def proj_k_m(xn, dn, m):
                k_fin(qk_psum("wkh", "wkl", xn, m), m, dn)

            def proj_v_t(t, n, borrow=False):
                """One [128 tok x 512 feature] tile of the v projection."""
                ps = proj_ps(borrow)
                dr_group(ps, xh_sb[:], xl_sb[:], w_sb["wvh"], w_sb["wvl"],
                         slice(t * 128, (t + 1) * 128),
                         slice(n * 512, (n + 1) * 512))
                nc.vector.tensor_scalar_mul(
                    v_v[:, t, n * 8:(n + 1) * 8, :DH],
                    ps[:].rearrange("p (h e) -> p h e", e=DH), WSCALE_INV)

            # ---- filler: PE work interleaved into the attention stream ----
            filler = []
            state = {"deficit": 0, "idx": 0}

            def add_filler(cycles, fn):
                filler.append((cycles, fn))

            def drain(cycles):
                state["deficit"] += cycles
                while (state["idx"] < len(filler)
                       and state["deficit"] >= filler[state["idx"]][0]):
                    cyc, fn = filler[state["idx"]]
                    state["idx"] += 1
                    state["deficit"] -= cyc
                    fn()

            def drain_to(idx):
                while state["idx"] < min(idx, len(filler)):
                    state["deficit"] = 0
                    cyc, fn = filler[state["idx"]]
                    state["idx"] += 1
                    fn()

            # ---- attention ----
            ctx_slabs = {}

            def ct8_split(src, kolo, kohi, csl):
                """fp8 hi/lo split of a transposed ctxT piece (the slab is
                already at 64x scale via the 1/64 AV ones-column)."""
                nc.vector.tensor_copy(ct8h_sb[:, kolo:kohi, csl], src)
                nc.vector.scalar_tensor_tensor(
                    ct8l_sb[:, kolo:kohi, csl], src, 1.0,
                    ct8h_sb[:, kolo:kohi, csl], ALU.mult, ALU.subtract)

            def transpose(c, qt):
                csl = slice((c * 4 + qt) * 128, (c * 4 + qt + 1) * 128)
                t = ctxtp.tile([128, 8, 128], bf16, tag="ctp", name="ctp")
                nc.sync.dma_start_transpose(t[:], ctx_slabs[(c, qt)][:])
                ct8_split(t[:], 0, 8, csl)

            KSPLIT = 4   # ko 0..KSPLIT-1 transposed early, rest in tail

            def transpose_half(c, qt, half):
                lo, hi = (0, KSPLIT) if half == 0 else (KSPLIT, 8)
                csl = slice((c * 4 + qt) * 128, (c * 4 + qt + 1) * 128)
                t = ctxtp.tile([128, 8, 128], bf16, tag="ctp", name="ctp")
                nc.sync.dma_start_transpose(
                    t[:, lo:hi, :],
                    ctx_slabs[(c, qt)][:, lo * 128:hi * 128])
                ct8_split(t[:, lo:hi, :], lo, hi, csl)

            def out_part(to, n):
                # dead bf16 storage for chunk-0 partials: n0 in the slab's
                # own lo half (read-complete after its early transpose), n1
                # in retired q8 m0/m1 slabs viewed as bf16.
                if n == 0:
                    return ctx_slabs[(0, to)][:, 0:512]
                m, half = divmod(to, 2)
                qv = q_sb[:, m, :, :].rearrange(
                    "p a b -> p (a b)").bitcast(bf16)
                return qv[:, half * 512:(half + 1) * 512]

            def out_mm(ps, to, n, ko0, ko1):
                """DR out-proj matmuls over ko tiles [ko0, ko1)."""
                tsl = slice(to * 128, (to + 1) * 128)
                nsl = slice(n * 512, (n + 1) * 512)
                steps = [(t, s) for t in range(3)
                         for s in range(ko0 // 2, ko1 // 2)]
                mats = ((ct8h_sb, wo8h_sb), (ct8h_sb, wo8l_sb),
                        (ct8l_sb, wo8h_sb))
                for i, (t, s) in enumerate(steps):
                    lh, rh = mats[t]
                    ksl = slice(2 * s, 2 * s + 2)
                    nc.tensor.matmul(
                        ps[:], lh[:, ksl, tsl], rh[:, ksl, nsl],
                        start=(i == 0), stop=(i == len(steps) - 1),
                        perf_mode=DR)

            def out_stage1(to, n):
                # chunk-0: accumulate ko 0..KSPLIT-1 mid-stream, stash the
                # bf16 partial (+bias) in dead storage
                ps = proj_ps()
                out_mm(ps, to, n, 0, KSPLIT)
                nc.vector.scalar_tensor_tensor(
                    out_part(to, n), ps[:], OUT_SCALE,
                    bo_sb[:, n * 512:(n + 1) * 512], ALU.mult, ALU.add)

            def out_stage2(to, n):
                ps = proj_ps()
                out_mm(ps, to, n, KSPLIT, 8)
                o_t = op.tile([128, 512], f32, tag="o", name="o")
                nc.vector.scalar_tensor_tensor(
                    o_t[:], ps[:], OUT_SCALE, out_part(to, n),
                    ALU.mult, ALU.add)
                nc.sync.dma_start(out_r[:, to, n * 512:(n + 1) * 512], o_t[:])

            PASSES = ((0, 0, 1), (1, 0, 1), (0, 2, 3), (1, 2, 3))
            pending = [None]      # (c, u, plist, slot, tiles) awaiting AV
            pair_no = [0]

            def av_pass(c, u, plist, g, qta, qtb):
                """One AV pass: heads 2u+g, query tiles (qta, qtb)."""
                h = 2 * u + g
                tiles = []
                for qt in (qta, qtb):
                    tiles.append(psV.tile([128, 512], f32, name="av",
                                          tag="av"))
                for kt in range(KT):
                    vsl = v_v[:, c * 4 + kt, h, :]
                    for qt, av in zip((qta, qtb), tiles):
                        nc.tensor.matmul(
                            av[:, 0:65],
                            plist[kt][:, g * 512 + qt * 128:
                                      g * 512 + (qt + 1) * 128],
                            vsl, start=(kt == 0), stop=(kt == KT - 1))
                for qt, av in zip((qta, qtb), tiles):
                    rec = recp.tile([128, 1], f32, tag="rec", name="rec")
                    nc.vector.reciprocal(rec[:], av[:, 64:65])
                    nc.vector.tensor_scalar_mul(
                        ctx_slabs[(c, qt)][:, h * 64:(h + 1) * 64],
                        av[:, 0:64], rec[:, 0:1])

            def flush_slot(s=None):
                """Emit AV pass s (or all remaining) of the pending pair;
                after the chunk's last pair, transposes chase the passes.
                All 4 passes land by kt3 so the pending p-tiles retire
                before the ptile pool (12 bufs) wraps at kt4."""
                if pending[0] is None:
                    return
                c, u, plist, done, _ = pending[0]
                rng = range(4) if s is None else [s]
                for i in rng:
                    if i < done:
                        continue
                    av_pass(c, u, plist, *PASSES[i])
                    pending[0] = (c, u, plist, i + 1, None)
                    if c == 0 and u == KSPLIT - 1 and i == 3:
                        for qt in range(4):
                            transpose_half(0, qt, 0)
                        for to in range(4):
                            add_filler(1536,
                                       lambda to=to: out_stage1(to, 0))
                        for to in range(4):
                            add_filler(1536,
                                       lambda to=to: out_stage1(to, 1))
                    if u == 7:
                        if i == 1:
                            if c == 0:
                                transpose_half(c, 0, 1)
                                transpose_half(c, 1, 1)
                            else:
                                transpose(c, 0)
                                transpose(c, 1)
                        if i == 3:
                            if c == 0:
                                transpose_half(c, 2, 1)
                                transpose_half(c, 3, 1)
                            else:
                                transpose(c, 2)
                                transpose(c, 3)
                if pending[0][3] >= 4:
                    pending[0] = None

            def emit_pair(c, u, force_idx=None):
                if force_idx is not None:
                    drain_to(force_idx)
                plist = []
                for kt in range(KT):
                    ksl = slice(c * 512 + kt * 128, c * 512 + (kt + 1) * 128)
                    qsl = slice(c * 512, (c + 1) * 512)
                    sps = psS.tile([128, 1024], f32, name="sps")
                    kA = k_sb[0:64, u, ksl].unsqueeze(1).broadcast_to(
                        [64, 2, 128])
                    kB = k_sb[64:128, u, ksl].unsqueeze(1).broadcast_to(
                        [64, 2, 128])
                    nc.tensor.matmul(sps[:, 0:512], kA,
                                     q_sb[0:64, u, :, qsl], start=True,
                                     stop=True, perf_mode=DR)
                    nc.tensor.matmul(sps[:, 512:1024], kB,
                                     q_sb[64:128, u, :, qsl], start=True,
                                     stop=True, perf_mode=DR)
                    p_t = pp.tile([128, 1024], bf16, tag="p", name="p")
                    if c == 0 and kt < 4:
                        nc.scalar.activation(p_t[:], sps[:], AF.Exp,
                                             bias=vb_sb[:, 0:1],
                                             scale=EXP_SCALE)
                    else:
                        nc.scalar.activation(p_t[:], sps[:], AF.Exp,
                                             scale=EXP_SCALE)
                    plist.append(p_t)
                    # drain below the exp-paced slack (2491 - 512 score -
                    # 520 avg AV cy/kt) so the PE can repay startup DMA
                    # debt; the excess PE work was front-loaded pre-stream.
                    budget = 1815
                    if kt < 4 and pending[0] is not None:
                        pc, pu = pending[0][0], pending[0][1]
                        if kt == 0 and (pc, pu) in av_force:
                            drain_to(av_force[(pc, pu)])
                        flush_slot(kt)
                        budget -= 1040
                    drain(max(budget, 0))
                flush_slot()   # no-op unless fewer than 8 slots were free
                pending[0] = (c, u, plist, 0, None)
                pair_no[0] += 1

            def out_proj(to, n, borrow=False):
                ps = proj_ps(borrow)
                out_mm(ps, to, n, 0, 8)
                o_t = op.tile([128, 512], f32, tag="o", name="o")
                nc.vector.scalar_tensor_tensor(
                    o_t[:], ps[:], OUT_SCALE,
                    bo_sb[:, n * 512:(n + 1) * 512], ALU.mult, ALU.add)
                nc.sync.dma_start(out_r[:, to, n * 512:(n + 1) * 512], o_t[:])

            # ---- phase schedule ----
            # Chunk 1 first: it touches no halo data, so the DMA stream
            # delivers own-token x/w first and the halo trails.
            nc.sync.dma_start(wo8h_sb[:], w_r["woh"][:])
            nc.sync.dma_start(wo8l_sb[:], w_r["wol"][:])

            # pre-attention: just what (1,0)'s first score tiles need,
            # as two interleaved m=0 groups ordered so the xl-dependent
            # cross terms come last (xl is the longest DMA pole).
            psQ = psA.tile([128, 512], f32, name="bps", tag="ps")
            psK = psV.tile([128, 512], f32, name="bps", tag="av")
            m0 = slice(0, 128)
            for ps, wh, xn in ((psQ, "wqh", 2), (psK, "wkh", 1)):
                xs = slice(xn * 512, (xn + 1) * 512)
                for s4 in range(4):
                    ks = slice(2 * s4, 2 * s4 + 2)
                    nc.tensor.matmul(ps[:], w_sb[wh][:, ks, m0],
                                     xh_sb[:, ks, xs],
                                     start=(s4 == 0), stop=False, perf_mode=DR)
            for ps, wl, xn in ((psQ, "wql", 2), (psK, "wkl", 1)):
                xs = slice(xn * 512, (xn + 1) * 512)
                for s4 in range(4):
                    ks = slice(2 * s4, 2 * s4 + 2)
                    nc.tensor.matmul(ps[:], w_sb[wl][:, ks, m0],
                                     xh_sb[:, ks, xs],
                                     start=False, stop=False, perf_mode=DR)
            for ps, wh, xn in ((psQ, "wqh", 2), (psK, "wkh", 1)):
                xs = slice(xn * 512, (xn + 1) * 512)
                for s4 in range(4):
                    ks = slice(2 * s4, 2 * s4 + 2)
                    nc.tensor.matmul(ps[:], w_sb[wh][:, ks, m0],
                                     xl_sb[:, ks, xs],
                                     start=False, stop=(s4 == 3), perf_mode=DR)
            q_fin(psQ, 0, 1)
            k_fin(psK, 0, 1)

            score_force = {}
            av_force = {}
            add_filler(3072, lambda: proj_k_m(2, 2, 0))

            def add_qk1(u):
                add_filler(3072, lambda m=u: proj_q_m(2, 1, m))
                add_filler(3072, lambda m=u: proj_k_m(1, 1, m))
                add_filler(3072, lambda m=u: proj_k_m(2, 2, m))
                score_force[(1, u)] = len(filler)

            def add_qk0(u):
                add_filler(3072, lambda m=u: proj_q_m(1, 0, m))
                add_filler(3072, lambda m=u: proj_k_m(0, 0, m))
                score_force[(0, u)] = len(filler)

            add_qk1(1)
            for t in range(4, 12):
                add_filler(3072, lambda t=t: proj_v_t(t, 0))
            av_force[(1, 0)] = len(filler)
            add_qk1(2)
            add_qk1(3)
            add_qk1(4)
            for t in range(4, 12):
                add_filler(3072, lambda t=t: proj_v_t(t, 1))
            av_force[(1, 4)] = len(filler)
            for u in (5, 6, 7):
                add_qk1(u)
            for u in range(8):
                add_qk0(u)
            for t in range(4):
                add_filler(3072, lambda t=t: proj_v_t(t, 0))
            av_force[(0, 0)] = len(filler)
            for t in range(4):
                add_filler(3072, lambda t=t: proj_v_t(t, 1))
            av_force[(0, 4)] = len(filler)
            for to in range(4, 8):
                for n in range(2):
                    add_filler(3072, lambda to=to, n=n: out_proj(to, n))

            # front-load the PE excess: k2-m0 + qk1(1) + the v-n0 batch
            # run inline pre-stream, paced by the DMA arrivals (the first
            # AV pass needs all eight v-n0 tiles and cannot wait for
            # in-stream draining).
            drain_to(12)

            for qt in range(4):
                ctx_slabs[(1, qt)] = ctxp.tile([128, BLK], bf16, tag="slab",
                                               name="slab")
            for u in range(8):
                emit_pair(1, u, force_idx=score_force.get((1, u)))
            for qt in range(4):
                ctx_slabs[(0, qt)] = ctxp.tile([128, BLK], bf16, tag="slab",
                                               name="slab")
            for u in range(8):
                emit_pair(0, u, force_idx=score_force.get((0, u)))

            flush_slot()
            drain_to(len(filler))
            for to in range(4):
                out_stage2(to, 0)
                out_stage2(to, 1)

    nc.compile()
    return nc


def _host_prep(x, Wq, bq, Wk, bk, Wv, bv, Wo, bo):
    import ml_dtypes

    e4 = ml_dtypes.float8_e4m3
    bf = ml_dtypes.bfloat16

    def split8(a):
        a = np.ascontiguousarray(a, dtype=np.float32)
        hi = a.astype(e4)
        lo = (a - hi.astype(np.float32)).astype(e4)
        return hi, lo

    x = np.ascontiguousarray(np.asarray(x, dtype=np.float32))
    Wq = np.asarray(Wq, np.float32)
    Wk = np.asarray(Wk, np.float32)
    Wv = np.asarray(Wv, np.float32)
    Wo = np.asarray(Wo, np.float32)
    bv = np.asarray(bv, np.float32)
    bo = np.asarray(bo, np.float32)

    # q/k weights at 32x so the fp8 q/k (psum scale) stay under e4m3's
    # 240 max; v/o weights at 64x as before.
    wqh, wql = split8(Wq.T * 32.0)
    wkh, wkl = split8(Wk.T * 32.0)
    wvh, wvl = split8(Wv.T * 64.0)
    woh, wol = split8(Wo.T * 64.0)
    bo_eff = bo + Wo @ bv          # v-bias folded through the attention avg
    mats = {
        "wqh": wqh, "wql": wql, "wkh": wkh, "wkl": wkl,
        "wvh": wvh, "wvl": wvl, "woh": woh, "wol": wol,
        "borep": np.ascontiguousarray(
            np.tile(bo_eff[None, :], (128, 1)).astype(bf)),
    }
    bqkv = np.zeros((128, 17), np.float32)
    # q bias at the 32x q scale; k bias is dropped (softmax-invariant).
    bqkv[:, 0:8] = np.asarray(bq, np.float32).reshape(8, 128).T * 32.0
    mats["bqkv_base"] = bqkv

    in_maps = []
    for core in range(NCORES):
        b, j = core // 4, core % 4
        start = j * BLK
        xkv = np.zeros((NKV, D), np.float32)
        lo = start - W
        if lo < 0:
            xkv[W:] = x[b, start:start + BLK]
        else:
            xkv[:] = x[b, lo:start + BLK]
        xh, xl = split8(xkv.T)
        im = dict(mats)
        bqkv_c = mats["bqkv_base"].copy()
        if j == 0:
            bqkv_c[:, 16] = -1e9   # chunk-0 halo k-tiles masked in the exp
        del im["bqkv_base"]
        im["bqkv"] = np.ascontiguousarray(bqkv_c)
        im["xhi"] = xh
        im["xlo"] = xl
        in_maps.append(im)
    return in_maps


def kernel(x, Wq, bq, Wk, bk, Wv, bv, Wo, bo):
    from concourse.bass_utils import run_bass_kernel_spmd

    if "nc" not in _CACHE:
        _CACHE["nc"] = _build()
    nc = _CACHE["nc"]

    in_maps = _host_prep(x, Wq, bq, Wk, bk, Wv, bv, Wo, bo)
    res = run_bass_kernel_spmd(nc, in_maps, list(range(NCORES)))

    out = np.empty((B, L, D), np.float32)
    for core in range(NCORES):
        b, j = core // 4, core % 4
        out[b, j * BLK:(j + 1) * BLK] = res.results[core]["out"]
    return out
